# revision 1
# baseline (speedup 1.0000x reference)
# Trainium2 Bass kernel for nn_MemoryBlock (topk_masking).
#
# Math (per batch b, per head h):
#   u  = log(relu(x)+1)
#   q  = target_token @ Wq.T + bq          (shared across batch)
#   kk = u @ Wk.T        (+bk skipped: rank-invariant per attention row)
#   v  = u @ Wv.T        (+bv folded into xo afterwards)
#   s  = q_h @ kk_h.T    (softmax+scale skipped: rank-invariant)
#   t64[g] = 64th largest of s[g, :]       (max8+match_replace chain)
#   mask = (s >= t64)                      (0/1, bf16)
#   xo_h = mask @ v_h / 64  (+bv)
#   global min/max over all cores (AllReduce), xo = exp((xo-mn)/(mx-mn))
#   out_b = xo @ Wout.T + bout
#
# Sharding: data parallel over batch (8 cores, one batch element each).
# Weights replicated; host pre-transposes weight matrices (layout marshaling
# only - all model compute runs on device).

import numpy as np

B, L, G, D, H = 8, 4096, 512, 512, 8
DH = D // H  # 64
KTOP = 64
NEG = -1e30

_CACHE = {}


def _concourse():
    try:
        import concourse.bass  # noqa: F401
    except ImportError:
        import sys
        for p in ("/opt/trn_rl_repo", "/root/.axon_site/_ro/trn_rl_repo"):
            if p not in sys.path:
                sys.path.insert(0, p)
    import concourse.bass as bass
    import concourse.mybir as mybir
    import concourse.tile as tile
    from concourse.masks import make_identity
    return bass, mybir, tile, make_identity


def build_program():
    bass, mybir, tile, make_identity = _concourse()
    from contextlib import ExitStack
    F32 = mybir.dt.float32
    BF16 = mybir.dt.bfloat16
    AX = mybir.AxisListType
    OP = mybir.AluOpType
    ACT = mybir.ActivationFunctionType

    from concourse import bacc
    # Bacc (not raw Bass): its compile() pass splits multi-wait sync into
    # event semaphores, which walrus codegen requires (1 wait/instruction).
    nc = bacc.Bacc("TRN2", num_devices=B)

    x_d = nc.declare_dram_parameter("x", [L, D], F32, isOutput=False)
    ttT_d = nc.declare_dram_parameter("ttT", [D, G], F32, isOutput=False)
    WqT_d = nc.declare_dram_parameter("WqT", [D, D], F32, isOutput=False)
    WkT_d = nc.declare_dram_parameter("WkT", [D, D], F32, isOutput=False)
    WvT_d = nc.declare_dram_parameter("WvT", [D, D], F32, isOutput=False)
    WoutT_d = nc.declare_dram_parameter("WoutT", [D, D], F32, isOutput=False)
    bq_d = nc.declare_dram_parameter("bq", [D], F32, isOutput=False)
    bv_d = nc.declare_dram_parameter("bv", [D], F32, isOutput=False)
    bout_d = nc.declare_dram_parameter("bout", [D], F32, isOutput=False)
    out_d = nc.declare_dram_parameter("out", [G, D], F32, isOutput=True)

    with tile.TileContext(nc) as tc, ExitStack() as top:
        pers = top.enter_context(tc.tile_pool(name="pers", bufs=1))

        ident = pers.tile([128, 128], F32)
        make_identity(nc, ident[:])

        # persistent operands (small)
        qT = pers.tile([128, 4, G], F32)        # q^T packed: [d, g]
        xoT = pers.tile([128, 4, G], F32)       # xo^T:        [d, g]
        bq_t = pers.tile([128, 4], F32)
        bv_t = pers.tile([128, 4], F32)
        nc.gpsimd.dma_start(out=bq_t[:], in_=bq_d[:].rearrange("(t p) -> p t", p=128))
        nc.gpsimd.dma_start(out=bv_t[:], in_=bv_d[:].rearrange("(t p) -> p t", p=128))
        brow = pers.tile([1, D], F32)
        nc.gpsimd.dma_start(out=brow[0:1, :], in_=bout_d[:].rearrange("(a d) -> a d", a=1))
        # ones row: K=1 matmul against this broadcasts a [1, N] row over
        # all 128 output partitions (avoids gpsimd library ops)
        ones_t = pers.tile([1, 128], F32)
        nc.vector.memset(ones_t[:], 1.0)

        # ---------------- phase A: u^T, q^T, kk^T, v ----------------
        # Pool open order (= reverse close order): kvpool (lives through
        # phase B) -> uTpool (lives to end of phase A) -> transient pools.
        stkKV = ExitStack()
        kvpool = stkKV.enter_context(tc.tile_pool(name="kvpool", bufs=1))
        kkT = kvpool.tile([128, 4, L], F32)      # kk^T packed: [d, j]
        vbf = kvpool.tile([128, 32, D], BF16)    # v natural:   [j, d]
        stkUT = ExitStack()
        uTpool = stkUT.enter_context(tc.tile_pool(name="uTpool", bufs=1))
        uT = uTpool.tile([128, 4, L], F32)

        with ExitStack() as phA:
            psA = phA.enter_context(tc.tile_pool(name="psA", bufs=4, space="PSUM"))

            with ExitStack() as phA1:
                upool = phA1.enter_context(tc.tile_pool(name="upool", bufs=1))
                xpool = phA1.enter_context(tc.tile_pool(name="xpool", bufs=2))

                # stream u in groups of 8 l-tiles; transpose each group into uT
                for lg in range(4):
                    u8 = upool.tile([128, 8, D], F32, tag="u8")
                    for lt8 in range(8):
                        lt = lg * 8 + lt8
                        xt = xpool.tile([128, D], F32, tag="xt")
                        wt = xpool.tile([128, D], F32, tag="wt")
                        nc.gpsimd.dma_start(out=xt[:], in_=x_d[lt * 128:(lt + 1) * 128, :])
                        nc.vector.tensor_scalar(wt[:], xt[:], 1.0, 1.0, op0=OP.add, op1=OP.max)
                        nc.scalar.activation(u8[:, lt8, :], wt[:], ACT.Ln)
                    for dt in range(4):
                        for pr in range(2):
                            pt = psA.tile([128, 512], F32, tag="psa")
                            for q4 in range(4):
                                lt8 = pr * 4 + q4
                                nc.tensor.transpose(
                                    pt[:, q4 * 128:(q4 + 1) * 128],
                                    u8[:, lt8, dt * 128:(dt + 1) * 128],
                                    ident[:],
                                )
                            nc.scalar.copy(
                                uT[:, dt, lg * 1024 + pr * 512:lg * 1024 + (pr + 1) * 512],
                                pt[:],
                            )

            # q^T = Wq @ tt^T + bq  (uses ttT, WqT; freed right after)
            with ExitStack() as phQ:
                wq_pool = phQ.enter_context(tc.tile_pool(name="wq_pool", bufs=1))
                WqT_t = wq_pool.tile([128, 4, D], F32)
                ttT_t = wq_pool.tile([128, 4, G], F32)
                for kt in range(4):
                    nc.gpsimd.dma_start(out=WqT_t[:, kt, :], in_=WqT_d[kt * 128:(kt + 1) * 128, :])
                    nc.gpsimd.dma_start(out=ttT_t[:, kt, :], in_=ttT_d[kt * 128:(kt + 1) * 128, :])
                for dt in range(4):
                    pq = psA.tile([128, 512], F32, tag="psa")
                    for kt in range(4):
                        nc.tensor.matmul(
                            pq[:], WqT_t[:, kt, dt * 128:(dt + 1) * 128], ttT_t[:, kt, :],
                            start=(kt == 0), stop=(kt == 3),
                        )
                    nc.vector.tensor_scalar(qT[:, dt, :], pq[:], bq_t[:, dt:dt + 1], None, op0=OP.add)

            # kk^T = Wk @ u^T
            with ExitStack() as phK:
                wk_pool = phK.enter_context(tc.tile_pool(name="wk_pool", bufs=1))
                WkT_t = wk_pool.tile([128, 4, D], F32)
                for kt in range(4):
                    nc.gpsimd.dma_start(out=WkT_t[:, kt, :], in_=WkT_d[kt * 128:(kt + 1) * 128, :])
                for dt in range(4):
                    for jc in range(8):
                        pk = psA.tile([128, 512], F32, tag="psa")
                        for kt in range(4):
                            nc.tensor.matmul(
                                pk[:], WkT_t[:, kt, dt * 128:(dt + 1) * 128],
                                uT[:, kt, jc * 512:(jc + 1) * 512],
                                start=(kt == 0), stop=(kt == 3),
                            )
                        nc.scalar.copy(kkT[:, dt, jc * 512:(jc + 1) * 512], pk[:])

            # v = u @ Wv^T (bf16, natural layout)
            with ExitStack() as phV:
                wv_pool = phV.enter_context(tc.tile_pool(name="wv_pool", bufs=1))
                WvT_t = wv_pool.tile([128, 4, D], F32)
                for kt in range(4):
                    nc.gpsimd.dma_start(out=WvT_t[:, kt, :], in_=WvT_d[kt * 128:(kt + 1) * 128, :])
                for lt in range(32):
                    pv = psA.tile([128, 512], F32, tag="psa")
                    for kt in range(4):
                        nc.tensor.matmul(
                            pv[:], uT[:, kt, lt * 128:(lt + 1) * 128], WvT_t[:, kt, :],
                            start=(kt == 0), stop=(kt == 3),
                        )
                    nc.scalar.copy(vbf[:, lt, :], pv[:])

        stkUT.close()  # uT no longer needed

        # ---------------- phase B: scores, top-64 threshold, mask, xo ----------------
        with ExitStack() as phB:
            spool = phB.enter_context(tc.tile_pool(name="spool", bufs=2))
            scpool = phB.enter_context(tc.tile_pool(name="scpool", bufs=1))
            mtpool = phB.enter_context(tc.tile_pool(name="mtpool", bufs=1))
            bpool = phB.enter_context(tc.tile_pool(name="bpool", bufs=2))
            trpool = phB.enter_context(tc.tile_pool(name="trpool", bufs=2))
            psS = phB.enter_context(tc.tile_pool(name="psS", bufs=4, space="PSUM"))
            psXO = phB.enter_context(tc.tile_pool(name="psXO", bufs=2, space="PSUM"))

            for h in range(H):
                hp = h // 2
                pb = (h % 2) * 64
                # --- per-row 64th largest (threshold) for all 4 g-tiles ---
                bvals = bpool.tile([128, 4, 64], F32, tag="bv")
                for gt in range(4):
                    s_t = spool.tile([128, L], F32, tag="s")
                    # scores s[g, j] for this (head, g-tile)
                    for jc in range(8):
                        ps = psS.tile([128, 512], F32, tag="ps")
                        nc.tensor.matmul(
                            ps[:],
                            qT[pb:pb + 64, hp, gt * 128:(gt + 1) * 128],
                            kkT[pb:pb + 64, hp, jc * 512:(jc + 1) * 512],
                            start=True, stop=True,
                        )
                        nc.scalar.copy(s_t[:, jc * 512:(jc + 1) * 512], ps[:])
                    # 64th-largest per row via max8 + match_replace chain
                    scratch = scpool.tile([128, L], F32, tag="scr")
                    for r in range(8):
                        src = s_t if r == 0 else scratch
                        nc.vector.max(out=bvals[:, gt, 8 * r:8 * r + 8], in_=src[:])
                        if r < 7:
                            nc.vector.match_replace(
                                out=scratch[:], in_to_replace=bvals[:, gt, 8 * r:8 * r + 8],
                                in_values=src[:], imm_value=NEG,
                            )
                # --- replicate thresholds to [128, g] via transpose + ones-matmul ---
                ptr = psS.tile([128, 512], F32, tag="ps")
                for gt in range(4):
                    nc.tensor.transpose(
                        ptr[0:1, gt * 128:(gt + 1) * 128],
                        bvals[:, gt, 63:64], ident[:],
                    )
                trow = trpool.tile([1, G], F32, tag="trow")
                nc.vector.tensor_copy(trow[0:1, :], ptr[0:1, :])
                ptr2 = psS.tile([128, 512], F32, tag="ps")
                nc.tensor.matmul(ptr2[:], ones_t[0:1, :], trow[0:1, :], start=True, stop=True)
                trep = trpool.tile([128, G], F32, tag="trep")
                nc.vector.tensor_copy(trep[:], ptr2[:])
                # --- mask^T[j, g] = (s^T >= t) via transposed-score recompute ---
                maskT_t = mtpool.tile([128, 32, G], BF16, tag="maskT")
                for jt in range(32):
                    pst = psS.tile([128, 512], F32, tag="ps")
                    nc.tensor.matmul(
                        pst[:],
                        kkT[pb:pb + 64, hp, jt * 128:(jt + 1) * 128],
                        qT[pb:pb + 64, hp, :],
                        start=True, stop=True,
                    )
                    nc.vector.tensor_tensor(
                        out=maskT_t[:, jt, :], in0=pst[:], in1=trep[:], op=OP.is_ge
                    )
                # --- xo^T_h = v_h^T @ mask^T / 64 + bv ---
                pxo = psXO.tile([64, G], F32, tag="pxo")
                for m in range(32):
                    nc.tensor.matmul(
                        pxo[:], vbf[:, m, h * DH:(h + 1) * DH], maskT_t[:, m, :],
                        start=(m == 0), stop=(m == 31),
                    )
                nc.vector.tensor_scalar(
                    xoT[pb:pb + 64, hp, :], pxo[:], 1.0 / KTOP, bv_t[pb:pb + 64, hp:hp + 1],
                    op0=OP.mult, op1=OP.add,
                )

        stkKV.close()  # kkT / vbf no longer needed

        # ---------------- phase C: global min/max, exp, out-projection ----------------
        with ExitStack() as phC:
            cpool = phC.enter_context(tc.tile_pool(name="cpool", bufs=1))
            dpool = phC.enter_context(tc.tile_pool(name="dpool", bufs=1, space="DRAM"))
            psC = phC.enter_context(tc.tile_pool(name="psC", bufs=4, space="PSUM"))

            rmx = cpool.tile([128, 4], F32)
            rmn = cpool.tile([128, 4], F32)
            for dt in range(4):
                nc.vector.tensor_reduce(out=rmx[:, dt:dt + 1], in_=xoT[:, dt, :], axis=AX.X, op=OP.max)
                nc.vector.tensor_reduce(out=rmn[:, dt:dt + 1], in_=xoT[:, dt, :], axis=AX.X, op=OP.min)
            mm2 = cpool.tile([128, 2], F32)
            nc.vector.tensor_reduce(out=mm2[:, 0:1], in_=rmx[:], axis=AX.X, op=OP.max)
            nc.vector.tensor_reduce(out=mm2[:, 1:2], in_=rmn[:], axis=AX.X, op=OP.min)
            nc.vector.tensor_scalar(mm2[:, 1:2], mm2[:, 1:2], -1.0, None, op0=OP.mult)
            mmtop = cpool.tile([1, 2], F32)
            nc.gpsimd.tensor_reduce(out=mmtop[:], in_=mm2[:], axis=AX.C, op=OP.max)

            cc_in = dpool.tile([1, 2], F32)
            cc_out = dpool.tile([1, 2], F32, addr_space="Shared")
            nc.gpsimd.dma_start(out=cc_in[:], in_=mmtop[:])
            nc.gpsimd.collective_compute(
                "AllReduce", OP.max,
                replica_groups=[list(range(B))],
                ins=[cc_in.opt()], outs=[cc_out.opt()],
            )
            gl = cpool.tile([1, 2], F32)
            nc.gpsimd.dma_start(out=gl[:], in_=cc_out[:])

            # scale = 1/(mx - mn), bias = -mn * scale (gl = [mx, -mn])
            rng_t = cpool.tile([1, 1], F32)
            nc.vector.tensor_tensor(out=rng_t[:], in0=gl[0:1, 0:1], in1=gl[0:1, 1:2], op=OP.add)
            sc2 = cpool.tile([1, 2], F32)
            nc.vector.reciprocal(sc2[0:1, 0:1], rng_t[:])
            nc.vector.tensor_tensor(out=sc2[0:1, 1:2], in0=gl[0:1, 1:2], in1=sc2[0:1, 0:1], op=OP.mult)
            # broadcast [1,2] -> [128,2] via K=1 matmul
            pb2 = psC.tile([128, 2], F32, tag="pb2")
            nc.tensor.matmul(pb2[:], ones_t[0:1, :], sc2[0:1, :], start=True, stop=True)
            sb2 = cpool.tile([128, 2], F32)
            nc.vector.tensor_copy(sb2[:], pb2[:])

            xon = cpool.tile([128, 4, G], F32)
            for dt in range(4):
                nc.scalar.activation(
                    xon[:, dt, :], xoT[:, dt, :], ACT.Exp,
                    bias=sb2[:, 1:2], scale=sb2[:, 0:1],
                )

            WoT_t = cpool.tile([128, 4, D], F32)
            for kt in range(4):
                nc.gpsimd.dma_start(out=WoT_t[:, kt, :], in_=WoutT_d[kt * 128:(kt + 1) * 128, :])
            for gt in range(4):
                po = psC.tile([128, D], F32, tag="po")
                for kt in range(4):
                    nc.tensor.matmul(
                        po[:], xon[:, kt, gt * 128:(gt + 1) * 128], WoT_t[:, kt, :],
                        start=(kt == 0), stop=False,
                    )
                # += bout broadcast over rows (K=1 ones matmul)
                nc.tensor.matmul(po[:], ones_t[0:1, :], brow[0:1, :], start=False, stop=True)
                ot = cpool.tile([128, D], F32, tag="ot", bufs=4)
                nc.vector.tensor_copy(ot[:], po[:])
                nc.gpsimd.dma_start(out=out_d[gt * 128:(gt + 1) * 128, :], in_=ot[:])

    nc.compile()
    return nc


def _get_exec():
    """Build + jit the 8-core SPMD executable once; cache for repeat calls."""
    if "exec" in _CACHE:
        return _CACHE["exec"]
    _concourse()
    import jax
    from jax.experimental.shard_map import shard_map
    from jax.sharding import Mesh, PartitionSpec
    import concourse.mybir as mybir
    from concourse import bass2jax

    nc = build_program()
    bass2jax.install_neuronx_cc_hook()

    in_names, out_names, out_avals, zero_shapes = [], [], [], []
    partition_name = nc.partition_id_tensor.name if nc.partition_id_tensor else None
    for alloc in nc.m.functions[0].allocations:
        if not isinstance(alloc, mybir.MemoryLocationSet):
            continue
        name = alloc.memorylocations[0].name
        if alloc.kind == "ExternalInput":
            if name != partition_name:
                in_names.append(name)
        elif alloc.kind == "ExternalOutput":
            shape = tuple(alloc.tensor_shape)
            dtype = mybir.dt.np(alloc.dtype)
            out_names.append(name)
            out_avals.append(jax.core.ShapedArray(shape, dtype))
            zero_shapes.append((shape, dtype))
    n_params = len(in_names)
    all_in_names = in_names + out_names
    if partition_name is not None:
        all_in_names = all_in_names + [partition_name]
    donate = tuple(range(n_params, n_params + len(out_names)))

    def _body(*args):
        operands = list(args)
        if partition_name is not None:
            operands.append(bass2jax.partition_id_tensor())
        outs = bass2jax._bass_exec_p.bind(
            *operands,
            out_avals=tuple(out_avals),
            in_names=tuple(all_in_names),
            out_names=tuple(out_names),
            lowering_input_output_aliases=(),
            sim_require_finite=True,
            sim_require_nnan=True,
            nc=nc,
        )
        return tuple(outs)

    devices = jax.devices()[:B]
    mesh = Mesh(np.asarray(devices), ("core",))
    specs_in = (PartitionSpec("core"),) * (n_params + len(out_names))
    specs_out = (PartitionSpec("core"),) * len(out_names)
    fn = jax.jit(
        shard_map(_body, mesh=mesh, in_specs=specs_in, out_specs=specs_out,
                  check_rep=False),
        donate_argnums=donate, keep_unused=True,
    )
    _CACHE["exec"] = (fn, in_names, out_names, zero_shapes, mesh)
    return _CACHE["exec"]


def _prep_inputs(inputs):
    """Host-side marshaling: shard x by batch, transpose weights, and
    concatenate per-core inputs along axis 0 (shard_map layout)."""
    f32c = lambda a: np.ascontiguousarray(np.asarray(a, dtype=np.float32))
    x = f32c(inputs["x"])
    shared = {
        "ttT": f32c(np.asarray(inputs["target_token"]).T),
        "WqT": f32c(np.asarray(inputs["Wq"]).T),
        "WkT": f32c(np.asarray(inputs["Wk"]).T),
        "WvT": f32c(np.asarray(inputs["Wv"]).T),
        "WoutT": f32c(np.asarray(inputs["Wout"]).T),
        "bq": f32c(inputs["bq"]),
        "bv": f32c(inputs["bv"]),
        "bout": f32c(inputs["bout"]),
    }
    per_core = [dict(shared, x=x[b]) for b in range(B)]
    _, in_names, _, _, _ = _get_exec()
    return [
        np.concatenate([per_core[c][nm] for c in range(B)], axis=0)
        for nm in in_names
    ]


def _zeros_outs():
    _, _, _, zero_shapes, _ = _get_exec()
    return [np.zeros((B * s[0], *s[1:]), dt) for (s, dt) in zero_shapes]


def kernel(**inputs):
    fn, in_names, out_names, zero_shapes, _ = _get_exec()
    concat_in = _prep_inputs(inputs)
    out_arrs = fn(*concat_in, *_zeros_outs())
    out = np.asarray(out_arrs[out_names.index("out")])
    return out.reshape(B, G, D)

